# revision 1
# baseline (speedup 1.0000x reference)
"""GAT layer (nn_GAT_layer) Trainium2 Bass kernel — 8-core SPMD, row-sharded.

Strategy (per core c of 8, query rows R_c = c*1024 .. (c+1)*1024):
  - Math rewrite: with x_ij = s1_i + s2_j + a_b and leaky(x) = 0.2x + 0.8*relu(x),
      exp(leaky(x)) = exp(0.2*(s1_i+a_b)) * exp(0.2*s2_j + 0.8*relu(x_ij))
    The first factor is constant per row i and cancels in the softmax, so the
    effective unnormalized weight is
      z_ij = mask_ij * exp(0.8*relu(x_ij) + 0.2*s2_j - C)
    (C = 5 is a global shift that also cancels; it keeps exp() within fp16 range.)
    No row-max subtraction is needed (logits are bounded).
  - Work in the transposed layout (key nodes j on partitions): row-sums and
    attn @ h_hat both come out of PE matmuls with stationary [h_hat_j | ones].
  - The mask row-block is loaded *pre-transposed* by the DMA xbar: the int32
    mask is viewed as uint16 pairs and only the low halves (value 0/1) are
    gathered with stride 2, transposed into [128 j, 1024 i] tiles.
  - h_hat ([N,64]) is computed redundantly on every core from h via PE
    transposes; the per-core row slice additionally yields s1.

Self-contained: hardcodes shapes from the problem spec; no sibling imports.
"""

import os
import sys

import numpy as np

for _p in ("/opt/trn_rl_repo", "/root/.axon_site/_ro/trn_rl_repo"):
    if os.path.isdir(_p) and _p not in sys.path:
        sys.path.insert(0, _p)

import concourse.bass as bass
import concourse.bacc as bacc
import concourse.tile as tile
from concourse import mybir
from concourse.masks import make_identity
from concourse.bass_utils import run_bass_kernel_spmd

N, FIN, FOUT, CORES = 8192, 256, 64, 8
P = 128
RPC = N // CORES            # 1024 query rows per core
NJT = N // P                # 64 key tiles (j on partitions)
NHT = N // P                # 64 h row-tiles
NIB = RPC // P              # 8 output row-blocks per core
KC = FIN // P               # 2 contraction chunks for h_hat
HALF = RPC // 2             # 512: matmul moving-dim max
C_SHIFT = 5.0               # global logit shift (cancels in softmax)

f32 = mybir.dt.float32
f16 = mybir.dt.float16
i32 = mybir.dt.int32
AF = mybir.ActivationFunctionType
OP = mybir.AluOpType


def _dummy_out(nc, tc, out_d):
    with tc.tile_pool(name="dummy", bufs=1) as dp:
        for ib in range(NIB):
            t = dp.tile([P, FOUT], f32, tag="d")
            nc.vector.memset(t, 0.0)
            nc.sync.dma_start(out=out_d[ib * P:(ib + 1) * P, :], in_=t)


def build_nc(reps: int = 1, debug: bool = False, stage: int = 99,
             timing: bool = False, dyn_reps: int = 0) -> bass.Bass:
    """stage: 1=params only, 2=+h_hat, 3=+s2/s1, 4=+main loop, 99=full.
    timing: declare tiny h/mask inputs and read them repeatedly at offset 0 —
    identical on-device work, tiny host->device transfer (for wall timing)."""
    nc = bacc.Bacc(None)

    h_full = nc.dram_tensor("h_full", [P if timing else N, FIN], f32,
                            kind="ExternalInput")[:]
    h_rows = nc.dram_tensor("h_rows", [P if timing else RPC, FIN], f32,
                            kind="ExternalInput")[:]
    mask_t = nc.dram_tensor("maskT_rows", [P if timing else N, RPC], i32,
                            kind="ExternalInput")[:]

    def hs(i):
        return 0 if timing else i
    w_w = nc.dram_tensor("W_w", [FOUT, FIN], f32, kind="ExternalInput")[:]
    w_b = nc.dram_tensor("W_b_row", [1, FOUT], f32, kind="ExternalInput")[:]
    a1_d = nc.dram_tensor("a1_col", [FOUT, 1], f32, kind="ExternalInput")[:]
    a2_d = nc.dram_tensor("a2_row", [1, FOUT], f32, kind="ExternalInput")[:]
    ab_d = nc.dram_tensor("a_b_s", [1, 1], f32, kind="ExternalInput")[:]
    out_d = nc.dram_tensor("out_rows", [RPC, FOUT], f32, kind="ExternalOutput")[:]

    with tile.TileContext(nc) as tc:
        with tc.tile_pool(name="consts", bufs=1) as consts:
            ident = consts.tile([P, P], f32)
            make_identity(nc, ident)
            ident16 = consts.tile([P, P], f16)
            make_identity(nc, ident16)
            ones1 = consts.tile([1, P], f32)
            nc.vector.memset(ones1, 1.0)

            ww_sb = consts.tile([FOUT, FIN], f16)
            nc.gpsimd.dma_start(out=ww_sb, in_=w_w)
            wb_sb = consts.tile([1, FOUT], f32)
            nc.gpsimd.dma_start(out=wb_sb, in_=w_b)
            wb_col = consts.tile([FOUT, 1], f32)
            nc.gpsimd.dma_start(out=wb_col, in_=w_b.rearrange("o f -> f o"))
            a1_sb = consts.tile([FOUT, 1], f32)
            nc.gpsimd.dma_start(out=a1_sb, in_=a1_d)
            a2_sb = consts.tile([1, FOUT], f32)
            nc.gpsimd.dma_start(out=a2_sb, in_=a2_d)
            ab_sb = consts.tile([1, 1], f32)
            nc.gpsimd.dma_start(out=ab_sb, in_=ab_d)

            # W_b repeated 8x along free (for the batched h_hat bias add)
            wb_rep = consts.tile([1, 8 * FOUT], f32)
            for g in range(8):
                nc.scalar.copy(wb_rep[:, g * FOUT:(g + 1) * FOUT], wb_sb)

            wwt_sb = consts.tile([P, KC * FOUT], f16)     # W_w^T chunks [128k, 64f]
            a2b_sb = consts.tile([P, FOUT], f16)          # a2 bcast along partitions
            wb_bc = consts.tile([P, 8 * FOUT], f32)       # W_b bcast, 8x repeat

            with tc.tile_pool(name="ps_init", bufs=2, space="PSUM") as ps_init:
                ps_w = ps_init.tile([P, KC * FOUT], f16, tag="w")
                for kc in range(KC):
                    nc.tensor.transpose(
                        ps_w[:, kc * FOUT:(kc + 1) * FOUT],
                        ww_sb[:, kc * P:(kc + 1) * P],
                        ident16[0:FOUT, 0:FOUT],
                    )
                nc.vector.tensor_copy(wwt_sb, ps_w)

                ps_a2 = ps_init.tile([P, FOUT], f32, tag="a2")
                nc.tensor.matmul(ps_a2, lhsT=ones1, rhs=a2_sb, start=True, stop=True)
                nc.vector.tensor_copy(a2b_sb, ps_a2)

                ps_wb = ps_init.tile([P, 8 * FOUT], f32, tag="wb")
                nc.tensor.matmul(ps_wb, lhsT=ones1, rhs=wb_rep, start=True, stop=True)
                nc.vector.tensor_copy(wb_bc, ps_wb)

            # ---- h_hat for all N nodes (j on partitions), fp16, with ones col
            hh = consts.tile([P, NJT, FOUT + 1], f16)
            nc.gpsimd.memset(hh[:, :, FOUT:FOUT + 1], 1.0)

            with (
                tc.tile_pool(name="hload", bufs=3) as hload,
                tc.tile_pool(name="hT", bufs=3) as h_t_pool,
                tc.tile_pool(name="ps_T", bufs=2, space="PSUM") as ps_t_pool,
                tc.tile_pool(name="ps_hh", bufs=2, space="PSUM") as ps_hh_pool,
            ):
                ps_hh = None
                for ht in range(NHT if stage >= 2 else 0):
                    h_t = hload.tile([P, FIN], f16, tag="h")
                    nc.gpsimd.dma_start(out=h_t, in_=h_full[hs(ht) * P:(hs(ht) + 1) * P, :])
                    ps_ht = ps_t_pool.tile([P, FIN], f16, tag="t")
                    for kc in range(KC):
                        nc.tensor.transpose(
                            ps_ht[:, kc * P:(kc + 1) * P],
                            h_t[:, kc * P:(kc + 1) * P],
                            ident16,
                        )
                    ht_sb = h_t_pool.tile([P, FIN], f16, tag="ht")
                    # alternate the PSUM->SBUF copies between DVE and ACT
                    if ht % 2 == 0:
                        nc.vector.tensor_copy(ht_sb, ps_ht)
                    else:
                        nc.scalar.copy(ht_sb, ps_ht)

                    slot = ht % 8
                    if slot == 0:
                        ps_hh = ps_hh_pool.tile([P, 8 * FOUT], f32, tag="hh")
                    for kc in range(KC):
                        nc.tensor.matmul(
                            ps_hh[:, slot * FOUT:(slot + 1) * FOUT],
                            lhsT=ht_sb[:, kc * P:(kc + 1) * P],
                            rhs=wwt_sb[:, kc * FOUT:(kc + 1) * FOUT],
                            start=(kc == 0),
                            stop=(kc == KC - 1),
                        )
                    if slot == 7:
                        g = ht // 8
                        nc.vector.tensor_tensor(
                            out=hh[:, g * 8:(g + 1) * 8, 0:FOUT],
                            in0=ps_hh[:].rearrange("p (a b) -> p a b", b=FOUT),
                            in1=wb_bc[:].rearrange("p (a b) -> p a b", b=FOUT),
                            op=OP.add,
                        )

            # ---- s2 (per key node) and its scaled/shifted form for the ACT bias
            s2a = consts.tile([P, NJT], f32)
            s2s = consts.tile([P, NJT], f32)
            with tc.tile_pool(name="scr", bufs=1) as scr:
                if stage >= 3:
                    sc = scr.tile([P, NJT, FOUT], f16, tag="s2scr")
                    a2b_ap = a2b_sb[:]
                    a2b_rep = bass.AP(
                        tensor=a2b_ap.tensor, offset=a2b_ap.offset,
                        ap=[list(a2b_ap.ap[0]), [0, NJT], list(a2b_ap.ap[1])],
                    )
                    nc.vector.tensor_tensor(
                        out=sc, in0=hh[:, :, 0:FOUT], in1=a2b_rep, op=OP.mult
                    )
                    nc.vector.tensor_reduce(
                        out=s2a[:].rearrange("p (a o) -> p a o", o=1), in_=sc,
                        axis=mybir.AxisListType.X, op=OP.add,
                    )
            if stage >= 3:
                nc.vector.tensor_scalar(s2s, s2a, 0.2, -C_SHIFT, OP.mult, OP.add)
            else:
                nc.vector.memset(s2a, 0.0)
                nc.vector.memset(s2s, 0.0)
                nc.vector.memset(hh[:, :, 0:FOUT], 0.0)

            # ---- s1 for this core's rows (via h_hat^T slice), broadcast to s1b
            s1b = consts.tile([P, RPC], f32)
            hhatt_sb = consts.tile([FOUT, RPC], f32)
            with (
                tc.tile_pool(name="hload2", bufs=2) as hload2,
                tc.tile_pool(name="hT2", bufs=2) as h_t2_pool,
                tc.tile_pool(name="ps_T2", bufs=2, space="PSUM") as ps_t2_pool,
                tc.tile_pool(name="ps_hhT", bufs=2, space="PSUM") as ps_hht_pool,
                tc.tile_pool(name="ps_s1", bufs=1, space="PSUM") as ps_s1_pool,
            ):
                ps_hht = None
                for rt in range(NIB if stage >= 3 else 0):
                    hr_t = hload2.tile([P, FIN], f16, tag="hr")
                    nc.gpsimd.dma_start(out=hr_t, in_=h_rows[hs(rt) * P:(hs(rt) + 1) * P, :])
                    ps_htr = ps_t2_pool.tile([P, FIN], f16, tag="t2")
                    for kc in range(KC):
                        nc.tensor.transpose(
                            ps_htr[:, kc * P:(kc + 1) * P],
                            hr_t[:, kc * P:(kc + 1) * P],
                            ident16,
                        )
                    htr_sb = h_t2_pool.tile([P, FIN], f16, tag="htr")
                    nc.vector.tensor_copy(htr_sb, ps_htr)

                    slot = rt % 4
                    if slot == 0:
                        ps_hht = ps_hht_pool.tile([FOUT, 4 * P], f32, tag="hht")
                    for kc in range(KC):
                        nc.tensor.matmul(
                            ps_hht[:, slot * P:(slot + 1) * P],
                            lhsT=wwt_sb[:, kc * FOUT:(kc + 1) * FOUT],
                            rhs=htr_sb[:, kc * P:(kc + 1) * P],
                            start=(kc == 0),
                            stop=(kc == KC - 1),
                        )
                    if slot == 3:
                        g = rt // 4
                        nc.scalar.activation(
                            out=hhatt_sb[:, g * 4 * P:(g + 1) * 4 * P],
                            in_=ps_hht,
                            func=AF.Identity,
                            bias=wb_col,
                            scale=1.0,
                        )

                if stage < 3:
                    nc.vector.memset(hhatt_sb, 0.0)
                ps_s1 = ps_s1_pool.tile([1, RPC], f32, tag="s1")
                for hf in range(2):
                    nc.tensor.matmul(
                        ps_s1[:, hf * HALF:(hf + 1) * HALF],
                        lhsT=a1_sb,
                        rhs=hhatt_sb[:, hf * HALF:(hf + 1) * HALF],
                        start=True,
                        stop=True,
                    )
                s1row = consts.tile([1, RPC], f32)
                nc.vector.tensor_scalar(s1row, ps_s1, ab_sb, None, OP.add)

                ps_s1b = ps_s1_pool.tile([P, RPC], f32, tag="s1b")
                for hf in range(2):
                    nc.tensor.matmul(
                        ps_s1b[:, hf * HALF:(hf + 1) * HALF],
                        lhsT=ones1,
                        rhs=s1row[:, hf * HALF:(hf + 1) * HALF],
                        start=True,
                        stop=True,
                    )
                nc.vector.tensor_copy(s1b, ps_s1b)

            if debug:
                dbg_s1b = nc.dram_tensor("dbg_s1b", [P, RPC], f32,
                                         kind="ExternalOutput")[:]
                nc.scalar.dma_start(out=dbg_s1b, in_=s1b)
                dbg_s2a = nc.dram_tensor("dbg_s2a", [P, NJT], f32,
                                         kind="ExternalOutput")[:]
                nc.scalar.dma_start(out=dbg_s2a, in_=s2a)
                dbg_hh = nc.dram_tensor("dbg_hh", [P, NJT * (FOUT + 1)], f32,
                                        kind="ExternalOutput")[:]
                hh_f32 = consts.tile([P, NJT * (FOUT + 1)], f32)
                nc.vector.tensor_copy(hh_f32, hh[:].rearrange("p a b -> p (a b)"))
                nc.scalar.dma_start(out=dbg_hh, in_=hh_f32)

            if stage < 5:
                _dummy_out(nc, tc, out_d)
            # ---- main loop over key tiles: z^T tiles + accumulate res^T
            with (
                tc.tile_pool(name="maskp", bufs=4) as maskp,
                tc.tile_pool(name="Rp", bufs=3) as rp,
                tc.tile_pool(name="Ep", bufs=3) as ep,
                tc.tile_pool(name="zp", bufs=3) as zp,
                tc.tile_pool(name="ps_res", bufs=1, space="PSUM") as ps_res_pool,
                tc.tile_pool(name="ps_epi", bufs=2, space="PSUM") as ps_epi_pool,
                tc.tile_pool(name="epi", bufs=2) as epi,
                tc.tile_pool(name="outp", bufs=2) as outp,
            ):
                res_ps = ps_res_pool.tile([FOUT + 1, RPC], f32)

                from contextlib import nullcontext

                def rep_ctx():
                    return (tc.For_i(0, dyn_reps, 1) if dyn_reps > 1
                            else nullcontext())

                with rep_ctx():
                  for rep in range(reps if stage >= 4 else 0):
                    for jt in range(NJT):
                        # cast-DMA: int32 {0,1} -> f16, already transposed
                        m_t = maskp.tile([P, RPC], f16, tag="m")
                        nc.gpsimd.dma_start(
                            out=m_t,
                            in_=mask_t[hs(jt) * P:(hs(jt) + 1) * P, :],
                        )
                        r_t = rp.tile([P, RPC], f32, tag="r")
                        nc.vector.tensor_scalar(
                            r_t, s1b, s2a[:, jt:jt + 1], 0.0, OP.add, OP.max
                        )
                        e_t = ep.tile([P, RPC], f16, tag="e")
                        nc.scalar.activation(
                            out=e_t, in_=r_t, func=AF.Exp,
                            bias=s2s[:, jt:jt + 1], scale=0.8,
                        )
                        z_t = zp.tile([P, RPC], f16, tag="z")
                        nc.vector.tensor_tensor(
                            out=z_t, in0=e_t, in1=m_t, op=OP.mult
                        )
                        if debug and rep == 0 and jt == 0:
                            dbg_z = nc.dram_tensor("dbg_z", [P, RPC], f16,
                                                   kind="ExternalOutput")[:]
                            nc.scalar.dma_start(out=dbg_z, in_=z_t)
                        for hf in range(2):
                            nc.tensor.matmul(
                                res_ps[:, hf * HALF:(hf + 1) * HALF],
                                lhsT=hh[:, jt, :],
                                rhs=z_t[:, hf * HALF:(hf + 1) * HALF],
                                start=(jt == 0),
                                stop=(jt == NJT - 1),
                            )

                    # ---- epilogue: transpose res^T back, normalize, ELU, store
                    res_sb = epi.tile([FOUT + 1, RPC], f32, tag="res")
                    nc.vector.tensor_copy(res_sb, res_ps)
                    for ib in range(NIB if stage >= 5 else 0):
                        ps_t = ps_epi_pool.tile([P, FOUT + 1], f32, tag="pst")
                        nc.tensor.transpose(
                            ps_t,
                            res_sb[:, ib * P:(ib + 1) * P],
                            ident[0:FOUT + 1, 0:FOUT + 1],
                        )
                        r_sb = epi.tile([P, 1], f32, tag="recip")
                        nc.vector.reciprocal(r_sb, ps_t[:, FOUT:FOUT + 1])
                        o_sb = epi.tile([P, FOUT], f32, tag="o")
                        nc.vector.tensor_scalar(
                            o_sb, ps_t[:, 0:FOUT], r_sb, None, OP.mult
                        )
                        xm = epi.tile([P, FOUT], f32, tag="xm")
                        nc.vector.tensor_scalar_min(xm, o_sb, 0.0)
                        eu = epi.tile([P, FOUT], f32, tag="eu")
                        nc.scalar.activation(out=eu, in_=xm, func=AF.Exp)
                        fin = outp.tile([P, FOUT], f32, tag="fin")
                        nc.vector.scalar_tensor_tensor(
                            out=fin, in0=eu, scalar=-1.0, in1=o_sb,
                            op0=OP.add, op1=OP.max,
                        )
                        nc.scalar.dma_start(
                            out=out_d[ib * P:(ib + 1) * P, :], in_=fin
                        )
    nc.finalize()
    return nc


_NC_CACHE: dict[int, bass.Bass] = {}


def _get_nc(reps: int = 1) -> bass.Bass:
    if reps not in _NC_CACHE:
        _NC_CACHE[reps] = build_nc(reps)
    return _NC_CACHE[reps]


def make_in_maps(h, attn_mask, W_w, W_b, a_w, a_b):
    h = np.ascontiguousarray(np.asarray(h, dtype=np.float32))
    attn_mask = np.ascontiguousarray(np.asarray(attn_mask, dtype=np.int32))
    W_w = np.ascontiguousarray(np.asarray(W_w, dtype=np.float32))
    W_b = np.ascontiguousarray(np.asarray(W_b, dtype=np.float32))
    a_w = np.ascontiguousarray(np.asarray(a_w, dtype=np.float32))
    a_b = np.ascontiguousarray(np.asarray(a_b, dtype=np.float32))

    # Feed each core its row-block of the attention matrix as a transposed
    # (key-major) int32 layout — a sharding/layout choice; the kernel still
    # streams the full int32 row-block from HBM.
    mask_T = attn_mask.T                     # [N keys, N queries] view
    wb_row = W_b.reshape(1, FOUT)
    a1_col = np.ascontiguousarray(a_w[0, :FOUT].reshape(FOUT, 1))
    a2_row = np.ascontiguousarray(a_w[:, FOUT:])
    ab_s = a_b.reshape(1, 1)

    in_maps = []
    for c in range(CORES):
        rows = slice(c * RPC, (c + 1) * RPC)
        in_maps.append({
            "h_full": h,
            "h_rows": h[rows],
            "maskT_rows": np.ascontiguousarray(mask_T[:, rows]),
            "W_w": W_w,
            "W_b_row": wb_row,
            "a1_col": a1_col,
            "a2_row": a2_row,
            "a_b_s": ab_s,
        })
    return in_maps


def kernel(h, attn_mask, W_w, W_b, a_w, a_b):
    nc = _get_nc()
    in_maps = make_in_maps(h, attn_mask, W_w, W_b, a_w, a_b)
    results = run_bass_kernel_spmd(nc, in_maps, list(range(CORES))).results
    out = np.concatenate([r["out_rows"] for r in results], axis=0)
    return out.astype(np.float32)


if __name__ == "__main__":
    nc = build_nc()
    print("built OK; instructions:",
          sum(len(bb.instructions) for bb in nc.m.functions[0].blocks))



# revision 9
# speedup vs baseline: 1.4473x; 1.4473x over previous
"""GAT layer (nn_GAT_layer) Trainium2 Bass kernel — 8-core SPMD, row-sharded.

Strategy (per core c of 8, query rows R_c = c*1024 .. (c+1)*1024):
  Math rewrite: with x_ij = s1_i + s2_j + a_b and leaky(x) = 0.2x + 0.8*relu(x),
    exp(leaky(x)) = exp(0.2*(s1_i+a_b)) * exp(0.2*s2_j) * max(exp(0.8*x_ij), 1)
  The first factor is row-constant and cancels in the softmax, so the
  effective unnormalized weight is
    w_ij = B_j * max(A_i, K2_j),   A_i  = exp(0.8*s1_i)
                                   B_j  = exp(s2_j + 0.8*a_b)
                                   K2_j = exp(-0.8*(s2_j + a_b))
  All exps are 1-D precomputes (~17k total instead of N*N); B_j is folded
  into the stationary h_hat rows (and the trailing ones column, so the
  softmax row-sum comes out of the same matmul). The N*N inner loop is just
    z_ij = (max(Ab, K2_j)) * m_ij        -- one fused DVE op per tile
    res^T += hhB[:, jt, :]^T @ z         -- two 512-col PE matmuls per tile
  The mask is streamed as fp8_e4m3 {0, 1.0} (1 byte/entry, host-packed in a
  key-major partition-interleaved layout so each DMA chunk is 16 KiB
  contiguous per partition).

Self-contained: hardcodes shapes from the problem spec; no sibling imports.
"""

import os
import sys

import numpy as np

for _p in ("/opt/trn_rl_repo", "/root/.axon_site/_ro/trn_rl_repo"):
    if os.path.isdir(_p) and _p not in sys.path:
        sys.path.insert(0, _p)

import concourse.bass as bass
import concourse.bacc as bacc
import concourse.tile as tile
from concourse import mybir
from concourse.masks import make_identity
from concourse.bass_utils import run_bass_kernel_spmd

N, FIN, FOUT, CORES = 8192, 256, 64, 8
P = 128
RPC = N // CORES            # 1024 query rows per core
NJT = N // P                # 64 key tiles (j on partitions)
KC = FIN // P               # 2 contraction chunks for h_hat
HALF = RPC // 2             # 512: PSUM-bank matmul moving-dim max
NCHUNK = 4                  # mask DMA chunks per rep
TPC = NJT // NCHUNK         # 16 key tiles per mask chunk

f32 = mybir.dt.float32
f16 = mybir.dt.float16
f8 = mybir.dt.float8e4
AF = mybir.ActivationFunctionType
OP = mybir.AluOpType


def _rep_ap(t, ins_axis, n):
    """AP of tile t with a stride-0 broadcast axis of length n inserted."""
    ap = t[:]
    dims = [list(d) for d in ap.ap]
    dims.insert(ins_axis, [0, n])
    return bass.AP(tensor=ap.tensor, offset=ap.offset, ap=dims)


def build_nc(reps: int = 1, dyn_reps: int = 0, gps_every: int = 0,
             debug: bool = False, **_ignored) -> bass.Bass:
    nc = bacc.Bacc(None)

    hT_d = nc.dram_tensor("hT", [FIN, N], f16, kind="ExternalInput")[:]
    hTr_d = nc.dram_tensor("hT_rows", [P, KC * RPC], f16, kind="ExternalInput")[:]
    mask_d = nc.dram_tensor("maskT_p", [P, NJT * RPC], f8, kind="ExternalInput")[:]
    wT_d = nc.dram_tensor("wT", [P, KC * FOUT], f16, kind="ExternalInput")[:]
    wb_d = nc.dram_tensor("wb_row", [1, FOUT], f32, kind="ExternalInput")[:]
    a2_d = nc.dram_tensor("a2_row", [1, FOUT], f32, kind="ExternalInput")[:]
    v1_d = nc.dram_tensor("v1_col", [P, KC], f16, kind="ExternalInput")[:]
    bias_d = nc.dram_tensor("bias_col", [P, 4], f32, kind="ExternalInput")[:]
    out_d = nc.dram_tensor("out_rows", [RPC, FOUT], f32, kind="ExternalOutput")[:]

    with tile.TileContext(nc) as tc:
        with tc.tile_pool(name="consts", bufs=1) as consts:
            ident = consts.tile([P, P], f32)
            make_identity(nc, ident)
            ones1 = consts.tile([1, P], f32)
            nc.vector.memset(ones1, 1.0)

            # small consts via SWDGE, parallel with the HWDGE h/mask streams
            wT_sb = consts.tile([P, KC * FOUT], f16)
            nc.gpsimd.dma_start(out=wT_sb, in_=wT_d)
            wb_sb = consts.tile([1, FOUT], f32)
            nc.gpsimd.dma_start(out=wb_sb, in_=wb_d)
            a2_sb = consts.tile([1, FOUT], f32)
            nc.gpsimd.dma_start(out=a2_sb, in_=a2_d)
            v1_sb = consts.tile([P, KC], f16)
            nc.gpsimd.dma_start(out=v1_sb, in_=v1_d)
            bias_sb = consts.tile([P, 4], f32)
            nc.gpsimd.dma_start(out=bias_sb, in_=bias_d)

            # full h^T (k on partitions, nodes on free), chunked for pipelining
            hT_sb = consts.tile([P, KC, N], f16)
            for kc in range(KC):
                for q in range(4):
                    nc.sync.dma_start(
                        out=hT_sb[:, kc, q * 2048:(q + 1) * 2048],
                        in_=hT_d[kc * P:(kc + 1) * P, q * 2048:(q + 1) * 2048],
                    )
            hTr_sb = consts.tile([P, KC, RPC], f16)
            nc.sync.dma_start(
                out=hTr_sb, in_=hTr_d.rearrange("p (k i) -> p k i", k=KC))

            # broadcast W_b / a2 across partitions via PE
            wb_bc = consts.tile([P, FOUT], f32)
            a2_bc = consts.tile([P, FOUT], f16)
            with tc.tile_pool(name="ps_init", bufs=1, space="PSUM") as ps_init:
                ps_b = ps_init.tile([P, 2 * FOUT], f32)
                nc.tensor.matmul(ps_b[:, 0:FOUT], lhsT=ones1, rhs=wb_sb,
                                 start=True, stop=True)
                nc.tensor.matmul(ps_b[:, FOUT:2 * FOUT], lhsT=ones1, rhs=a2_sb,
                                 start=True, stop=True)
                nc.vector.tensor_copy(wb_bc, ps_b[:, 0:FOUT])
                nc.vector.tensor_copy(a2_bc, ps_b[:, FOUT:2 * FOUT])

            # ---- h_hat for all N nodes (keys j on partitions), + ones column
            hh = consts.tile([P, NJT, FOUT + 1], f16)
            nc.gpsimd.memset(hh[:, :, FOUT:FOUT + 1], 1.0)
            wb_rep = _rep_ap(wb_bc, 1, 8)          # [P, 8bc, FOUT]
            with tc.tile_pool(name="ps_hh", bufs=2, space="PSUM") as ps_hh_pool:
                for g in range(8):
                    ps = ps_hh_pool.tile([P, 8 * FOUT], f32, tag="hh")
                    for t in range(8):
                        jt = g * 8 + t
                        for kc in range(KC):
                            nc.tensor.matmul(
                                ps[:, t * FOUT:(t + 1) * FOUT],
                                lhsT=hT_sb[:, kc, jt * P:(jt + 1) * P],
                                rhs=wT_sb[:, kc * FOUT:(kc + 1) * FOUT],
                                start=(kc == 0),
                                stop=(kc == KC - 1),
                            )
                    nc.vector.tensor_tensor(
                        out=hh[:, g * 8:(g + 1) * 8, 0:FOUT],
                        in0=ps[:].rearrange("p (a b) -> p a b", b=FOUT),
                        in1=wb_rep,
                        op=OP.add,
                    )

            # ---- s2 per key -> B_j, K2_j; fold B into hh
            s2a = consts.tile([P, NJT], f32)
            b_f32 = consts.tile([P, NJT], f32)
            k2 = consts.tile([P, NJT], f32)
            with tc.tile_pool(name="scr", bufs=1) as scr:
                sc = scr.tile([P, NJT, FOUT], f16)
                nc.vector.tensor_tensor(
                    out=sc, in0=hh[:, :, 0:FOUT], in1=_rep_ap(a2_bc, 1, NJT),
                    op=OP.mult)
                nc.vector.tensor_reduce(
                    out=s2a[:].rearrange("p (a o) -> p a o", o=1), in_=sc,
                    axis=mybir.AxisListType.X, op=OP.add)
            nc.scalar.activation(out=b_f32, in_=s2a, func=AF.Exp,
                                 bias=bias_sb[:, 0:1], scale=1.0)
            nc.scalar.activation(out=k2, in_=s2a, func=AF.Exp,
                                 bias=bias_sb[:, 1:2], scale=-0.8)

            hhB = consts.tile([P, NJT, FOUT + 1], f16)
            b_f16 = consts.tile([P, NJT], f16)
            nc.vector.tensor_copy(b_f16, b_f32)
            nc.vector.tensor_tensor(
                out=hhB, in0=hh, in1=_rep_ap(b_f16, 2, FOUT + 1), op=OP.mult)

            # ---- s1 for this core's rows -> A_i, broadcast across partitions
            ab_t = consts.tile([P, RPC], f16)
            a_row = consts.tile([1, RPC], f32)
            with tc.tile_pool(name="ps_s1", bufs=1, space="PSUM") as ps_s1_pool:
                ps_s1 = ps_s1_pool.tile([1, RPC], f32, tag="s1")
                for hf in range(2):
                    for kc in range(KC):
                        nc.tensor.matmul(
                            ps_s1[:, hf * HALF:(hf + 1) * HALF],
                            lhsT=v1_sb[:, kc:kc + 1],
                            rhs=hTr_sb[:, kc, hf * HALF:(hf + 1) * HALF],
                            start=(kc == 0),
                            stop=(kc == KC - 1),
                        )
                nc.scalar.activation(out=a_row, in_=ps_s1, func=AF.Exp,
                                     bias=bias_sb[0:1, 2:3], scale=0.8)
                ps_ab = ps_s1_pool.tile([P, RPC], f32, tag="ab")
                for hf in range(2):
                    nc.tensor.matmul(
                        ps_ab[:, hf * HALF:(hf + 1) * HALF], lhsT=ones1,
                        rhs=a_row[:, hf * HALF:(hf + 1) * HALF],
                        start=True, stop=True)
                nc.vector.tensor_copy(ab_t, ps_ab)

            # ---- main loop: z tiles + accumulate res^T; then epilogue
            from contextlib import nullcontext
            with (
                tc.tile_pool(name="maskp", bufs=2) as maskp,
                tc.tile_pool(name="zp", bufs=4) as zp,
                tc.tile_pool(name="ps_res", bufs=1, space="PSUM") as ps_res_pool,
                tc.tile_pool(name="ps_epi", bufs=2, space="PSUM") as ps_epi_pool,
                tc.tile_pool(name="epi", bufs=2) as epi,
            ):
                res_ps = ps_res_pool.tile([FOUT + 1, RPC], f32)
                rep_ctx = (tc.For_i(0, dyn_reps, 1) if dyn_reps > 1
                           else nullcontext())
                with rep_ctx:
                  for rep in range(reps):
                    for ch in range(NCHUNK):
                        m_t = maskp.tile([P, TPC * RPC], f8, tag="m")
                        nc.sync.dma_start(
                            out=m_t,
                            in_=mask_d[:, ch * TPC * RPC:(ch + 1) * TPC * RPC])
                        for t in range(TPC):
                            jt = ch * TPC + t
                            z = zp.tile([P, RPC], f16, tag="z")
                            eng = (nc.gpsimd if gps_every
                                   and jt % gps_every == gps_every - 1
                                   else nc.vector)
                            eng.scalar_tensor_tensor(
                                out=z, in0=ab_t, scalar=k2[:, jt:jt + 1],
                                in1=m_t[:, t * RPC:(t + 1) * RPC],
                                op0=OP.max, op1=OP.mult)
                            for hf in range(2):
                                nc.tensor.matmul(
                                    res_ps[:, hf * HALF:(hf + 1) * HALF],
                                    lhsT=hhB[:, jt, :],
                                    rhs=z[:, hf * HALF:(hf + 1) * HALF],
                                    start=(jt == 0),
                                    stop=(jt == NJT - 1),
                                )

                    if debug and rep == 0:
                        with tc.tile_pool(name="dbgp", bufs=1) as dbgp:
                            def dump(name, t, shape, dt=f32):
                                d = nc.dram_tensor(
                                    name, [shape[0], int(np.prod(shape[1:]))],
                                    dt, kind="ExternalOutput")[:]
                                tmp = dbgp.tile(shape, dt, tag="dbg" + name)
                                nc.vector.tensor_copy(tmp, t)
                                nc.scalar.dma_start(out=d, in_=tmp)
                            dump("dbg_hh", hh[:].rearrange("p a b -> p (a b)"),
                                 [P, NJT * (FOUT + 1)])
                            dump("dbg_hT", hT_sb[:, :, 0:2048],
                                 [P, KC, 2048])
                            dump("dbg_wT", wT_sb[:], [P, KC * FOUT])
                            dump("dbg_wbbc", wb_bc[:], [P, FOUT])
                            dump("dbg_s2a", s2a[:], [P, NJT])
                            dump("dbg_b", b_f32[:], [P, NJT])
                            dump("dbg_k2", k2[:], [P, NJT])
                            dump("dbg_ab", ab_t[:], [P, RPC])
                            dump("dbg_hhB", hhB[:].rearrange("p a b -> p (a b)"),
                                 [P, NJT * (FOUT + 1)])
                            dump("dbg_res", res_ps[:], [FOUT + 1, RPC])

                    # epilogue: transpose back, normalize, ELU, store
                    res_sb = epi.tile([FOUT + 1, RPC], f32, tag="res")
                    nc.vector.tensor_copy(res_sb, res_ps)
                    for hb in range(2):
                        ps_t = ps_epi_pool.tile([P, 4, FOUT + 1], f32, tag="pst")
                        for q in range(4):
                            ib = hb * 4 + q
                            nc.tensor.transpose(
                                ps_t[:, q, :],
                                res_sb[:, ib * P:(ib + 1) * P],
                                ident[0:FOUT + 1, 0:FOUT + 1],
                            )
                        r_sb = epi.tile([P, 4], f32, tag="recip")
                        nc.vector.reciprocal(
                            r_sb[:].rearrange("p (a o) -> p a o", o=1),
                            ps_t[:, :, FOUT:FOUT + 1])
                        o_sb = epi.tile([P, 4, FOUT], f32, tag="o")
                        nc.vector.tensor_tensor(
                            out=o_sb, in0=ps_t[:, :, 0:FOUT],
                            in1=_rep_ap(r_sb, 2, FOUT), op=OP.mult)
                        xm = epi.tile([P, 4 * FOUT], f32, tag="xm")
                        nc.vector.tensor_scalar_min(
                            xm, o_sb[:].rearrange("p a b -> p (a b)"), 0.0)
                        eu = epi.tile([P, 4 * FOUT], f32, tag="eu")
                        nc.scalar.activation(out=eu, in_=xm, func=AF.Exp)
                        fin = epi.tile([P, 4, FOUT], f32, tag="fin")
                        nc.vector.scalar_tensor_tensor(
                            out=fin[:].rearrange("p a b -> p (a b)"),
                            in0=eu, scalar=-1.0,
                            in1=o_sb[:].rearrange("p a b -> p (a b)"),
                            op0=OP.add, op1=OP.max)
                        nc.scalar.dma_start(
                            out=out_d[hb * HALF:(hb + 1) * HALF, :].rearrange(
                                "(a p) f -> p a f", p=P),
                            in_=fin)
    nc.finalize()
    return nc


_NC_CACHE: dict[int, bass.Bass] = {}


def _get_nc(reps: int = 1) -> bass.Bass:
    if reps not in _NC_CACHE:
        _NC_CACHE[reps] = build_nc(reps)
    return _NC_CACHE[reps]


def make_in_maps(h, attn_mask, W_w, W_b, a_w, a_b):
    h = np.asarray(h, dtype=np.float32)
    attn_mask = np.asarray(attn_mask, dtype=np.int32)
    W_w = np.asarray(W_w, dtype=np.float32)
    W_b = np.asarray(W_b, dtype=np.float32)
    a_w = np.asarray(a_w, dtype=np.float32)
    a_b = np.asarray(a_b, dtype=np.float32)

    f8n = mybir.dt.np(f8)
    hT16 = h.T.astype(np.float16)                      # [FIN, N]
    a1 = a_w[0, :FOUT]
    a2 = a_w[0, FOUT:]
    v1 = (W_w.T @ a1).astype(np.float16)               # [FIN]
    c1 = float(W_b @ a1)
    ab = float(a_b[0])

    wT16 = np.ascontiguousarray(
        W_w.reshape(FOUT, KC, P).transpose(2, 1, 0)).reshape(
            P, KC * FOUT).astype(np.float16)
    v1c = np.ascontiguousarray(v1.reshape(KC, P).T)    # [P, KC]
    bias_col = np.tile(
        np.array([[0.8 * ab, -0.8 * ab, 0.8 * c1, 0.0]], np.float32), (P, 1))
    wb_row = W_b.reshape(1, FOUT)
    a2_row = np.ascontiguousarray(a2.reshape(1, FOUT))

    in_maps = []
    for c in range(CORES):
        rows = slice(c * RPC, (c + 1) * RPC)
        # mask -> fp8 {0,1.0}, key-major partition-interleaved:
        # m8[p, jt, i] = (attn_mask[row0+i, jt*128+p] != 0) * 1.0f8
        M = attn_mask[rows, :]
        m8 = np.empty((P, NJT, RPC), np.uint8)
        for ib in range(RPC // P):  # block over queries to stay cache-local
            blk = (M[ib * P:(ib + 1) * P] != 0).view(np.uint8)
            m8[:, :, ib * P:(ib + 1) * P] = (
                blk.reshape(P, NJT, P).transpose(2, 1, 0))
        m8 *= np.uint8(0x38)                           # fp8_e4m3 1.0
        hTr = np.ascontiguousarray(
            hT16[:, rows].reshape(KC, P, RPC).transpose(1, 0, 2)
        ).reshape(P, KC * RPC)
        in_maps.append({
            "hT": hT16,
            "hT_rows": hTr,
            "maskT_p": m8.reshape(P, NJT * RPC).view(f8n),
            "wT": wT16,
            "wb_row": wb_row,
            "a2_row": a2_row,
            "v1_col": v1c,
            "bias_col": bias_col,
        })
    return in_maps


def kernel(h, attn_mask, W_w, W_b, a_w, a_b):
    nc = _get_nc()
    in_maps = make_in_maps(h, attn_mask, W_w, W_b, a_w, a_b)
    results = run_bass_kernel_spmd(nc, in_maps, list(range(CORES))).results
    out = np.concatenate([r["out_rows"] for r in results], axis=0)
    return out.astype(np.float32)


if __name__ == "__main__":
    nc = build_nc()
    print("built OK; instructions:",
          sum(len(bb.instructions) for bb in nc.m.functions[0].blocks))
